# revision 11
# baseline (speedup 1.0000x reference)
"""Trainium2 Bass kernel for nn_LinearWindowToFilters.

Pipeline per core (data-parallel over batch, 8 cores):
  x[16384,1024] -> 5x (Linear -> BatchNorm(batch stats) -> ReLU) -> Linear -> sigmoid
  -> Bernoulli sampling against precomputed jax uniforms -> (chosen_events, log_probs)

Layout: features on partitions, batch on the free axis ("transposed" layout).
The host pre-transposes inputs when sharding so the device does zero transposes.
BatchNorm stats are computed with the DVE bn_stats/bn_aggr ops per shard and
combined across cores with one tiny AllReduce per BN layer (5 total).
The BN scale (gamma/sigma > 0) is folded into the next layer's weights;
the BN shift becomes a per-partition bias of a fused relu(z + b) op.
"""

import os
import sys

import numpy as np

sys.path.insert(0, "/opt/trn_rl_repo")

B = 131072
W = 64
E = 16
F = W * E  # 1024
NCORES = 8
BC = B // NCORES  # 16384 rows per core
NB = 512  # batch tile (free-dim) size
NT = BC // NB  # 32 tiles per core
HID = [130, 110, 90, 70, 50]  # hidden dims (BN layers 0..4)
EPS = 1e-5

_prog_cache = {}
last_results = None  # BassKernelResults of the most recent run (for test harness)


def _build_program():
    import concourse.bacc as bacc
    import concourse.bass as bass
    import concourse.mybir as mybir
    import concourse.tile as tile

    f32 = mybir.dt.float32
    AF = mybir.ActivationFunctionType
    OP = mybir.AluOpType

    nc = bacc.Bacc(
        "TRN2",
        target_bir_lowering=False,
        debug=False,
        num_devices=NCORES,
    )

    # ---------------- I/O ----------------
    xT = nc.dram_tensor("xT", [F, BC], f32, kind="ExternalInput")
    uT = nc.dram_tensor("uT", [W, BC], f32, kind="ExternalInput")
    w0T = nc.dram_tensor("w0T", [F, HID[0]], f32, kind="ExternalInput")
    wiT = [
        nc.dram_tensor(f"w{i}T", [HID[i - 1], HID[i]], f32, kind="ExternalInput")
        for i in range(1, 5)
    ]
    w5T = nc.dram_tensor("w5T", [HID[4], W], f32, kind="ExternalInput")
    b5d = nc.dram_tensor("b5", [W], f32, kind="ExternalInput")
    gd = [nc.dram_tensor(f"g{i}", [HID[i]], f32, kind="ExternalInput") for i in range(5)]
    bed = [
        nc.dram_tensor(f"be{i}", [HID[i]], f32, kind="ExternalInput") for i in range(5)
    ]
    chosenT = nc.dram_tensor("chosenT", [W, BC], f32, kind="ExternalOutput")
    logp = nc.dram_tensor("logp", [1, BC], f32, kind="ExternalOutput")

    rg = [list(range(NCORES))]

    with tile.TileContext(nc) as tc:
        with (
            tc.tile_pool(name="wpool", bufs=1) as wp,
            tc.tile_pool(name="dram", bufs=1, space="DRAM") as dp,
            tc.tile_pool(name="psum", bufs=3, space="PSUM") as pp,
        ):
            # ---------- load weights ----------
            w0a_sb = []
            w0b_sb = []
            for c in range(8):
                wa = wp.tile([128, 128], f32, name=f"w0a_{c}", tag=f"w0a_{c}")
                nc.sync.dma_start(wa, w0T.ap()[c * 128 : (c + 1) * 128, 0:128])
                w0a_sb.append(wa)
                wb = wp.tile([128, 2], f32, name=f"w0b_{c}", tag=f"w0b_{c}")
                nc.sync.dma_start(wb, w0T.ap()[c * 128 : (c + 1) * 128, 128:130])
                w0b_sb.append(wb)
            # raw next-layer weights (to be BN-scale-folded at runtime)
            w1a_raw = wp.tile([128, HID[1]], f32, name="w1a_raw", tag="w1a_raw")
            nc.sync.dma_start(w1a_raw, wiT[0].ap()[0:128, :])
            w1b_raw = wp.tile([2, HID[1]], f32, name="w1b_raw", tag="w1b_raw")
            nc.sync.dma_start(w1b_raw, wiT[0].ap()[128:130, :])
            w_raw = [None, None]
            for i in range(2, 5):
                wr = wp.tile([HID[i - 1], HID[i]], f32, name=f"w{i}_raw", tag=f"w{i}_raw")
                nc.sync.dma_start(wr, wiT[i - 1].ap()[:, :])
                w_raw.append(wr)
            w5_raw = wp.tile([HID[4], W], f32, name="w5_raw", tag="w5_raw")
            nc.sync.dma_start(w5_raw, w5T.ap()[:, :])

            # folded weights
            wf1a = wp.tile([128, HID[1]], f32, name="wf1a", tag="wf1a")
            wf1b = wp.tile([2, HID[1]], f32, name="wf1b", tag="wf1b")
            wf = [None, None]
            for i in range(2, 5):
                wf.append(wp.tile([HID[i - 1], HID[i]], f32, name=f"wf{i}", tag=f"wf{i}"))
            wf5 = wp.tile([HID[4], W], f32, name="wf5", tag="wf5")

            # gamma/beta/b5 vectors
            g_sb, be_sb = [], []
            for i in range(5):
                d = HID[i]
                da = min(d, 128)
                gt = wp.tile([da, 1], f32, name=f"g{i}a", tag=f"g{i}a")
                nc.sync.dma_start(gt, gd[i].ap()[0:da][:, None])
                bt = wp.tile([da, 1], f32, name=f"be{i}a", tag=f"be{i}a")
                nc.sync.dma_start(bt, bed[i].ap()[0:da][:, None])
                git = wp.tile([da, 1], f32, name=f"gi{i}a", tag=f"gi{i}a")
                nc.vector.reciprocal(git, gt)
                g_sb.append((gt, git))
                be_sb.append(bt)
            g0b = wp.tile([2, 1], f32, name="g0b", tag="g0b")
            nc.sync.dma_start(g0b, gd[0].ap()[128:130][:, None])
            gi0b = wp.tile([2, 1], f32, name="gi0b", tag="gi0b")
            nc.vector.reciprocal(gi0b, g0b)
            be0b = wp.tile([2, 1], f32, name="be0b", tag="be0b")
            nc.sync.dma_start(be0b, bed[0].ap()[128:130][:, None])
            b5neg = wp.tile([W, 1], f32, name="b5neg", tag="b5neg")
            nc.sync.dma_start(b5neg, b5d.ap()[:, None])
            nc.vector.tensor_scalar_mul(b5neg, b5neg, -1.0)

            # 0.5 (not 1.0): the log-sum matmul consumes log(d^2) = 2*log|d|
            ones_sb = wp.tile([W, 1], f32, name="ones_sb", tag="ones_sb")
            nc.vector.memset(ones_sb, 0.5)

            # bn stats accumulation buffers: [D, 6*NT]
            bn0a = wp.tile([128, 6 * NT], f32, name="bn0a", tag="bn0a")
            bn0b = wp.tile([2, 6 * NT], f32, name="bn0b", tag="bn0b")
            bn_buf = [None]
            for i in range(1, 5):
                bn_buf.append(
                    wp.tile([HID[i], 6 * NT], f32, name=f"bn{i}", tag=f"bn{i}")
                )

            # per-layer stat scratch tiles
            def stat_tiles(dpart, nm):
                return {
                    "st": wp.tile([dpart, 2], f32, name=f"st_{nm}", tag=f"st_{nm}"),
                    "pay": wp.tile([dpart, 2], f32, name=f"pay_{nm}", tag=f"pay_{nm}"),
                    "gst": wp.tile([dpart, 2], f32, name=f"gst_{nm}", tag=f"gst_{nm}"),
                    "t0": wp.tile([dpart, 1], f32, name=f"t0_{nm}", tag=f"t0_{nm}"),
                    "t1": wp.tile([dpart, 1], f32, name=f"t1_{nm}", tag=f"t1_{nm}"),
                    "mu": wp.tile([dpart, 1], f32, name=f"mu_{nm}", tag=f"mu_{nm}"),
                    "sd": wp.tile([dpart, 1], f32, name=f"sd_{nm}", tag=f"sd_{nm}"),
                    "a": wp.tile([dpart, 1], f32, name=f"a_{nm}", tag=f"a_{nm}"),
                    "nb": wp.tile([dpart, 1], f32, name=f"nb_{nm}", tag=f"nb_{nm}"),
                }

            sti = [stat_tiles(128, "0a")] + [stat_tiles(HID[i], str(i)) for i in range(1, 5)]
            st0b = stat_tiles(2, "0b")

            # z0b round-trip dram buffer
            z0b_dram = dp.tile([NT, 2, NB], f32, name="z0b_dram", tag="z0b_dram")

            # AR bounce buffers per layer
            ar_in = [
                dp.tile([HID[i], 2], f32, name=f"ar_in{i}", tag=f"ar_in{i}")
                for i in range(5)
            ]
            ar_out = [
                dp.tile([HID[i], 2], f32, name=f"ar_out{i}", tag=f"ar_out{i}")
                for i in range(5)
            ]

            def compute_payload(s, bnb, nslice):
                # aggregate bn stats -> (mean, var); payload = (mean, var + mean^2)
                nc.vector.bn_aggr(s["st"], bnb[:, 0 : 6 * NT])
                nc.vector.tensor_tensor(
                    s["t0"], s["st"][:, 0:1], s["st"][:, 0:1], op=OP.mult
                )
                nc.vector.tensor_copy(s["pay"][:, 0:1], s["st"][:, 0:1])
                nc.vector.tensor_tensor(
                    s["pay"][:, 1:2], s["st"][:, 1:2], s["t0"], op=OP.add
                )
                nc.sync.dma_start(nslice, s["pay"])

            def postprocess_stats(s, g_ap, ginv_ap, be_ap, oslice):
                # gst = sum over cores of (mean_c, E_c[z^2]); all [D,1] per-partition math
                nc.sync.dma_start(s["gst"], oslice)
                nc.vector.tensor_scalar_mul(s["mu"], s["gst"][:, 0:1], 1.0 / NCORES)
                nc.vector.tensor_scalar_mul(s["t0"], s["gst"][:, 1:2], 1.0 / NCORES)
                nc.vector.tensor_tensor(s["t1"], s["mu"], s["mu"], op=OP.mult)
                nc.vector.tensor_tensor(s["t0"], s["t0"], s["t1"], op=OP.subtract)
                # sd = sqrt(var + eps)
                nc.vector.tensor_scalar_add(s["t0"], s["t0"], EPS)
                nc.scalar.activation(s["sd"], s["t0"], AF.Sqrt)
                nc.vector.reciprocal(s["t1"], s["sd"])  # 1/sd
                nc.vector.tensor_tensor(s["a"], s["t1"], g_ap, op=OP.mult)
                # nb = be*sd/g - mu
                nc.vector.tensor_tensor(s["t0"], be_ap, s["sd"], op=OP.mult)
                nc.vector.tensor_tensor(s["t0"], s["t0"], ginv_ap, op=OP.mult)
                nc.vector.tensor_tensor(s["nb"], s["t0"], s["mu"], op=OP.subtract)

            with tc.tile_pool(name="zA", bufs=NT) as zA, tc.tile_pool(
                name="zB", bufs=NT
            ) as zB:
                # ================= phase 0: x -> z0 =================
                z0a_tiles = []
                z0b_scr = []
                with tc.tile_pool(name="xp", bufs=2) as xp:
                    for t in range(NT):
                        xt = xp.tile([128, 8 * NB], f32, name=f"x_{t}", tag="x")
                        src = xT.ap()[:, t * NB : (t + 1) * NB].rearrange(
                            "(c p) j -> p c j", p=128
                        )
                        nc.sync.dma_start(xt, src)
                        pa = pp.tile([128, NB], f32, name=f"p0a_{t}", tag="pa")
                        pb = pp.tile([2, NB], f32, name=f"p0b_{t}", tag="pb")
                        for c in range(8):
                            nc.tensor.matmul(
                                pa,
                                w0a_sb[c],
                                xt[:, c * NB : (c + 1) * NB],
                                start=(c == 0),
                                stop=(c == 7),
                            )
                        for c in range(8):
                            nc.tensor.matmul(
                                pb,
                                w0b_sb[c],
                                xt[:, c * NB : (c + 1) * NB],
                                start=(c == 0),
                                stop=(c == 7),
                            )
                        za = zA.tile([128, NB], f32, name=f"z0a_{t}", tag="zA")
                        nc.scalar.copy(za, pa)
                        z0a_tiles.append(za)
                        nc.vector.bn_stats(bn0a[:, 6 * t : 6 * t + 6], pa)
                        zb = xp.tile([2, NB], f32, name=f"z0b_{t}", tag="z0bs", bufs=3)
                        nc.scalar.copy(zb, pb)
                        nc.vector.bn_stats(bn0b[:, 6 * t : 6 * t + 6], pb)
                        nc.sync.dma_start(z0b_dram[t], zb)

                    # stats + AllReduce for layer 0
                    compute_payload(sti[0], bn0a, ar_in[0][0:128, :])
                    compute_payload(st0b, bn0b, ar_in[0][128:130, :])
                    nc.gpsimd.collective_compute(
                        "AllReduce",
                        OP.add,
                        replica_groups=rg,
                        ins=[ar_in[0].opt()],
                        outs=[ar_out[0].opt()],
                    )
                    postprocess_stats(
                        sti[0], g_sb[0][0], g_sb[0][1], be_sb[0], ar_out[0][0:128, :]
                    )
                    postprocess_stats(st0b, g0b, gi0b, be0b, ar_out[0][128:130, :])
                    # fold BN scale into W1
                    nc.vector.tensor_scalar_mul(wf1a, w1a_raw, sti[0]["a"])
                    nc.vector.tensor_scalar_mul(wf1b, w1b_raw, st0b["a"])

                # ================= phases 1..4 =================
                prev_a = z0a_tiles
                cur_tiles = None
                for li in range(1, 5):
                    d = HID[li]
                    zp = zB if li % 2 == 1 else zA
                    ztag = "zB" if li % 2 == 1 else "zA"
                    cur_tiles = []
                    for t in range(NT):
                        # relu(z + nb) in place on prev layer tile
                        if li == 1:
                            nc.vector.tensor_scalar(
                                prev_a[t],
                                prev_a[t],
                                sti[0]["nb"],
                                0.0,
                                op0=OP.add,
                                op1=OP.max,
                            )
                            hb = wp.tile(
                                [2, NB], f32, name=f"h0b_{t}", tag="h0b", bufs=4
                            )
                            nc.sync.dma_start(hb, z0b_dram[t])
                            nc.vector.tensor_scalar(
                                hb, hb, st0b["nb"], 0.0, op0=OP.add, op1=OP.max
                            )
                            ps = pp.tile([d, NB], f32, name=f"p1_{t}", tag="pa")
                            nc.tensor.matmul(ps, wf1a, prev_a[t], start=True, stop=False)
                            nc.tensor.matmul(ps, wf1b, hb, start=False, stop=True)
                        else:
                            nc.vector.tensor_scalar(
                                prev_a[t],
                                prev_a[t],
                                sti[li - 1]["nb"],
                                0.0,
                                op0=OP.add,
                                op1=OP.max,
                            )
                            ps = pp.tile([d, NB], f32, name=f"p{li}_{t}", tag="pa")
                            nc.tensor.matmul(
                                ps, wf[li], prev_a[t], start=True, stop=True
                            )
                        zt = zp.tile([d, NB], f32, name=f"z{li}_{t}", tag=ztag)
                        nc.scalar.copy(zt, ps)
                        cur_tiles.append(zt)
                        nc.vector.bn_stats(bn_buf[li][:, 6 * t : 6 * t + 6], ps)

                    compute_payload(sti[li], bn_buf[li], ar_in[li][:, :])
                    nc.gpsimd.collective_compute(
                        "AllReduce",
                        OP.add,
                        replica_groups=rg,
                        ins=[ar_in[li].opt()],
                        outs=[ar_out[li].opt()],
                    )
                    postprocess_stats(
                        sti[li], g_sb[li][0], g_sb[li][1], be_sb[li], ar_out[li][:, :]
                    )
                    if li < 4:
                        nc.vector.tensor_scalar_mul(wf[li + 1], w_raw[li + 1], sti[li]["a"])
                    else:
                        nc.vector.tensor_scalar_mul(wf5, w5_raw, sti[4]["a"])
                    prev_a = cur_tiles

                # ================= phase 5: final linear + sampling =================
                with tc.tile_pool(name="fp", bufs=3) as fp:
                    for t in range(NT):
                        nc.vector.tensor_scalar(
                            prev_a[t],
                            prev_a[t],
                            sti[4]["nb"],
                            0.0,
                            op0=OP.add,
                            op1=OP.max,
                        )
                        ps = pp.tile([W, NB], f32, name=f"p5_{t}", tag="pa")
                        nc.tensor.matmul(ps, wf5, prev_a[t], start=True, stop=True)
                        ut = fp.tile([W, NB], f32, name=f"u_{t}", tag="u")
                        nc.sync.dma_start(ut, uT.ap()[:, t * NB : (t + 1) * NB])
                        q = fp.tile([W, NB], f32, name=f"q_{t}", tag="q")
                        # q = sigmoid(-(z5 + b5)) = 1 - p
                        nc.scalar.activation(q, ps, AF.Sigmoid, bias=b5neg, scale=-1.0)
                        ce = fp.tile([W, NB], f32, name=f"ce_{t}", tag="ce")
                        nc.vector.tensor_tensor(ce, ut, q, op=OP.is_ge)
                        nc.sync.dma_start(chosenT.ap()[:, t * NB : (t + 1) * NB], ce)
                        dd = fp.tile([W, NB], f32, name=f"d_{t}", tag="d")
                        nc.vector.tensor_tensor(dd, ce, q, op=OP.subtract)
                        nc.vector.tensor_tensor(dd, dd, dd, op=OP.mult)
                        lg = fp.tile([W, NB], f32, name=f"lg_{t}", tag="lg")
                        nc.scalar.activation(lg, dd, AF.Ln)
                        pl = pp.tile([1, NB], f32, name=f"pl_{t}", tag="pb")
                        nc.tensor.matmul(pl, ones_sb, lg, start=True, stop=True)
                        lp = fp.tile([1, NB], f32, name=f"lp_{t}", tag="lp")
                        nc.scalar.copy(lp, pl)
                        nc.sync.dma_start(logp.ap()[:, t * NB : (t + 1) * NB], lp)

    nc.compile()
    return nc


def _get_program():
    if "nc" not in _prog_cache:
        _prog_cache["nc"] = _build_program()
    return _prog_cache["nc"]


def _compute_uniforms():
    import jax
    import jax.numpy as jnp

    cpu = jax.devices("cpu")[0]
    with jax.default_device(cpu):
        u = jax.random.uniform(jax.random.key(42), (B, W), dtype=jnp.float32)
        return np.asarray(u)


def kernel(**inputs):
    global last_results
    from concourse import bass_utils

    events = np.ascontiguousarray(np.asarray(inputs["events"], dtype=np.float32))
    x = events.reshape(B, F)
    u = _compute_uniforms()

    weights = {}
    weights["w0T"] = np.ascontiguousarray(np.asarray(inputs["W0"], np.float32).T)
    for i in range(1, 5):
        weights[f"w{i}T"] = np.ascontiguousarray(
            np.asarray(inputs[f"W{i}"], np.float32).T
        )
    weights["w5T"] = np.ascontiguousarray(np.asarray(inputs["W5"], np.float32).T)
    weights["b5"] = np.ascontiguousarray(np.asarray(inputs["b5"], np.float32))
    for i in range(5):
        weights[f"g{i}"] = np.ascontiguousarray(np.asarray(inputs[f"g{i}"], np.float32))
        weights[f"be{i}"] = np.ascontiguousarray(
            np.asarray(inputs[f"be{i}"], np.float32)
        )

    in_maps = []
    for c in range(NCORES):
        m = dict(weights)
        m["xT"] = np.ascontiguousarray(x[c * BC : (c + 1) * BC, :].T)
        m["uT"] = np.ascontiguousarray(u[c * BC : (c + 1) * BC, :].T)
        in_maps.append(m)

    nc = _get_program()
    trace = os.environ.get("KERNEL_TRACE", "0") == "1"
    res = bass_utils.run_bass_kernel_spmd(
        nc,
        in_maps,
        core_ids=list(range(NCORES)),
        trace=trace,
    )
    last_results = res

    chosen = np.empty((B, W), dtype=np.float32)
    log_probs = np.empty((B,), dtype=np.float32)
    for c in range(NCORES):
        r = res.results[c]
        chosen[c * BC : (c + 1) * BC, :] = r["chosenT"].T
        log_probs[c * BC : (c + 1) * BC] = r["logp"].reshape(-1)
    return chosen, log_probs


# revision 13
# speedup vs baseline: 1.5317x; 1.5317x over previous
"""Trainium2 Bass kernel for nn_LinearWindowToFilters.

Pipeline per core (data-parallel over batch, 8 cores):
  x[16384,1024] -> 5x (Linear -> BatchNorm(batch stats) -> ReLU) -> Linear -> sigmoid
  -> Bernoulli sampling against precomputed jax uniforms -> (chosen_events, log_probs)

Layout: features on partitions, batch on the free axis ("transposed" layout).
The host pre-transposes inputs when sharding so the device does zero transposes.
BatchNorm stats are computed with the DVE bn_stats/bn_aggr ops per shard and
combined across cores with one tiny AllReduce per BN layer (5 total).
The BN scale (gamma/sigma > 0) is folded into the next layer's weights;
the BN shift becomes a per-partition bias of a fused relu(z + b) op.
"""

import os
import sys

import numpy as np

sys.path.insert(0, "/opt/trn_rl_repo")

B = 131072
W = 64
E = 16
F = W * E  # 1024
NCORES = 8
BC = B // NCORES  # 16384 rows per core
NB = 512  # batch tile (free-dim) size
NT = BC // NB  # 32 tiles per core
HID = [130, 110, 90, 70, 50]  # hidden dims (BN layers 0..4)
EPS = 1e-5

_prog_cache = {}
last_results = None  # BassKernelResults of the most recent run (for test harness)


def _build_program():
    import concourse.bacc as bacc
    import concourse.bass as bass
    import concourse.mybir as mybir
    import concourse.tile as tile

    f32 = mybir.dt.float32
    f32r = mybir.dt.float32r
    AF = mybir.ActivationFunctionType
    OP = mybir.AluOpType

    nc = bacc.Bacc(
        "TRN2",
        target_bir_lowering=False,
        debug=False,
        num_devices=NCORES,
    )

    # ---------------- I/O ----------------
    xT = nc.dram_tensor("xT", [F, BC], f32r, kind="ExternalInput")
    uT = nc.dram_tensor("uT", [W, BC], f32, kind="ExternalInput")
    w0T = nc.dram_tensor("w0T", [F, HID[0]], f32r, kind="ExternalInput")
    wiT = [
        nc.dram_tensor(f"w{i}T", [HID[i - 1], HID[i]], f32, kind="ExternalInput")
        for i in range(1, 5)
    ]
    w5T = nc.dram_tensor("w5T", [HID[4], W], f32, kind="ExternalInput")
    b5d = nc.dram_tensor("b5", [W], f32, kind="ExternalInput")
    gd = [nc.dram_tensor(f"g{i}", [HID[i]], f32, kind="ExternalInput") for i in range(5)]
    bed = [
        nc.dram_tensor(f"be{i}", [HID[i]], f32, kind="ExternalInput") for i in range(5)
    ]
    chosenT = nc.dram_tensor("chosenT", [W, BC], f32, kind="ExternalOutput")
    logp = nc.dram_tensor("logp", [1, BC], f32, kind="ExternalOutput")

    rg = [list(range(NCORES))]

    with tile.TileContext(nc) as tc:
        with (
            tc.tile_pool(name="wpool", bufs=1) as wp,
            tc.tile_pool(name="dram", bufs=1, space="DRAM") as dp,
            tc.tile_pool(name="psum", bufs=3, space="PSUM") as pp,
        ):
            # ---------- load weights ----------
            w0a_sb = []
            w0b_sb = []
            for c in range(8):
                wa = wp.tile([128, 128], f32r, name=f"w0a_{c}", tag=f"w0a_{c}")
                nc.sync.dma_start(wa, w0T.ap()[c * 128 : (c + 1) * 128, 0:128])
                w0a_sb.append(wa)
                wb = wp.tile([128, 2], f32r, name=f"w0b_{c}", tag=f"w0b_{c}")
                nc.sync.dma_start(wb, w0T.ap()[c * 128 : (c + 1) * 128, 128:130])
                w0b_sb.append(wb)
            # raw next-layer weights (to be BN-scale-folded at runtime)
            w1a_raw = wp.tile([128, HID[1]], f32, name="w1a_raw", tag="w1a_raw")
            nc.sync.dma_start(w1a_raw, wiT[0].ap()[0:128, :])
            w1b_raw = wp.tile([2, HID[1]], f32, name="w1b_raw", tag="w1b_raw")
            nc.sync.dma_start(w1b_raw, wiT[0].ap()[128:130, :])
            w_raw = [None, None]
            for i in range(2, 5):
                wr = wp.tile([HID[i - 1], HID[i]], f32, name=f"w{i}_raw", tag=f"w{i}_raw")
                nc.sync.dma_start(wr, wiT[i - 1].ap()[:, :])
                w_raw.append(wr)
            w5_raw = wp.tile([HID[4], W], f32, name="w5_raw", tag="w5_raw")
            nc.sync.dma_start(w5_raw, w5T.ap()[:, :])

            # folded weights
            wf1a = wp.tile([128, HID[1]], f32r, name="wf1a", tag="wf1a")
            wf1b = wp.tile([2, HID[1]], f32r, name="wf1b", tag="wf1b")
            wf = [None, None]
            for i in range(2, 5):
                wf.append(wp.tile([HID[i - 1], HID[i]], f32r, name=f"wf{i}", tag=f"wf{i}"))
            wf5 = wp.tile([HID[4], W], f32r, name="wf5", tag="wf5")

            # gamma/beta/b5 vectors
            g_sb, be_sb = [], []
            for i in range(5):
                d = HID[i]
                da = min(d, 128)
                gt = wp.tile([da, 1], f32, name=f"g{i}a", tag=f"g{i}a")
                nc.sync.dma_start(gt, gd[i].ap()[0:da][:, None])
                bt = wp.tile([da, 1], f32, name=f"be{i}a", tag=f"be{i}a")
                nc.sync.dma_start(bt, bed[i].ap()[0:da][:, None])
                git = wp.tile([da, 1], f32, name=f"gi{i}a", tag=f"gi{i}a")
                nc.vector.reciprocal(git, gt)
                g_sb.append((gt, git))
                be_sb.append(bt)
            g0b = wp.tile([2, 1], f32, name="g0b", tag="g0b")
            nc.sync.dma_start(g0b, gd[0].ap()[128:130][:, None])
            gi0b = wp.tile([2, 1], f32, name="gi0b", tag="gi0b")
            nc.vector.reciprocal(gi0b, g0b)
            be0b = wp.tile([2, 1], f32, name="be0b", tag="be0b")
            nc.sync.dma_start(be0b, bed[0].ap()[128:130][:, None])
            b5neg = wp.tile([W, 1], f32, name="b5neg", tag="b5neg")
            nc.sync.dma_start(b5neg, b5d.ap()[:, None])
            nc.vector.tensor_scalar_mul(b5neg, b5neg, -1.0)

            # 0.5 (not 1.0): the log-sum matmul consumes log(d^2) = 2*log|d|
            ones_f = wp.tile([W, 1], f32, name="ones_f", tag="ones_f")
            nc.vector.memset(ones_f, 0.5)
            ones_sb = wp.tile([W, 1], f32r, name="ones_sb", tag="ones_sb")
            nc.vector.tensor_scalar(ones_sb, ones_f, 0.0, None, op0=OP.add)

            # bn stats accumulation buffers: [D, 6*NT]
            bn0a = wp.tile([128, 6 * NT], f32, name="bn0a", tag="bn0a")
            bn0b = wp.tile([2, 6 * NT], f32, name="bn0b", tag="bn0b")
            bn_buf = [None]
            for i in range(1, 5):
                bn_buf.append(
                    wp.tile([HID[i], 6 * NT], f32, name=f"bn{i}", tag=f"bn{i}")
                )

            # per-layer stat scratch tiles
            def stat_tiles(dpart, nm):
                return {
                    "st": wp.tile([dpart, 2], f32, name=f"st_{nm}", tag=f"st_{nm}"),
                    "pay": wp.tile([dpart, 2], f32, name=f"pay_{nm}", tag=f"pay_{nm}"),
                    "gst": wp.tile([dpart, 2], f32, name=f"gst_{nm}", tag=f"gst_{nm}"),
                    "t0": wp.tile([dpart, 1], f32, name=f"t0_{nm}", tag=f"t0_{nm}"),
                    "t1": wp.tile([dpart, 1], f32, name=f"t1_{nm}", tag=f"t1_{nm}"),
                    "mu": wp.tile([dpart, 1], f32, name=f"mu_{nm}", tag=f"mu_{nm}"),
                    "sd": wp.tile([dpart, 1], f32, name=f"sd_{nm}", tag=f"sd_{nm}"),
                    "a": wp.tile([dpart, 1], f32, name=f"a_{nm}", tag=f"a_{nm}"),
                    "nb": wp.tile([dpart, 1], f32, name=f"nb_{nm}", tag=f"nb_{nm}"),
                }

            sti = [stat_tiles(128, "0a")] + [stat_tiles(HID[i], str(i)) for i in range(1, 5)]
            st0b = stat_tiles(2, "0b")

            # z0b round-trip dram buffer
            z0b_dram = dp.tile([NT, 2, NB], f32r, name="z0b_dram", tag="z0b_dram")

            # AR bounce buffers per layer
            ar_in = [
                dp.tile([HID[i], 2], f32, name=f"ar_in{i}", tag=f"ar_in{i}")
                for i in range(5)
            ]
            ar_out = [
                dp.tile([HID[i], 2], f32, name=f"ar_out{i}", tag=f"ar_out{i}")
                for i in range(5)
            ]

            def compute_payload(s, bnb, nslice):
                # aggregate bn stats -> (mean, var); payload = (mean, var + mean^2)
                nc.vector.bn_aggr(s["st"], bnb[:, 0 : 6 * NT])
                nc.vector.tensor_tensor(
                    s["t0"], s["st"][:, 0:1], s["st"][:, 0:1], op=OP.mult
                )
                nc.vector.tensor_copy(s["pay"][:, 0:1], s["st"][:, 0:1])
                nc.vector.tensor_tensor(
                    s["pay"][:, 1:2], s["st"][:, 1:2], s["t0"], op=OP.add
                )
                nc.sync.dma_start(nslice, s["pay"])

            def postprocess_stats(s, g_ap, ginv_ap, be_ap, oslice):
                # gst = sum over cores of (mean_c, E_c[z^2]); all [D,1] per-partition math
                nc.sync.dma_start(s["gst"], oslice)
                nc.vector.tensor_scalar_mul(s["mu"], s["gst"][:, 0:1], 1.0 / NCORES)
                nc.vector.tensor_scalar_mul(s["t0"], s["gst"][:, 1:2], 1.0 / NCORES)
                nc.vector.tensor_tensor(s["t1"], s["mu"], s["mu"], op=OP.mult)
                nc.vector.tensor_tensor(s["t0"], s["t0"], s["t1"], op=OP.subtract)
                # sd = sqrt(var + eps)
                nc.vector.tensor_scalar_add(s["t0"], s["t0"], EPS)
                nc.scalar.activation(s["sd"], s["t0"], AF.Sqrt)
                nc.vector.reciprocal(s["t1"], s["sd"])  # 1/sd
                nc.vector.tensor_tensor(s["a"], s["t1"], g_ap, op=OP.mult)
                # nb = be*sd/g - mu
                nc.vector.tensor_tensor(s["t0"], be_ap, s["sd"], op=OP.mult)
                nc.vector.tensor_tensor(s["t0"], s["t0"], ginv_ap, op=OP.mult)
                nc.vector.tensor_tensor(s["nb"], s["t0"], s["mu"], op=OP.subtract)

            with tc.tile_pool(name="zA", bufs=NT) as zA, tc.tile_pool(
                name="zB", bufs=NT
            ) as zB:
                # ================= phase 0: x -> z0 =================
                z0a_tiles = []
                z0b_scr = []
                with tc.tile_pool(name="xp", bufs=2) as xp:
                    for t in range(NT):
                        xt = xp.tile([128, 8 * NB], f32r, name=f"x_{t}", tag="x")
                        src = xT.ap()[:, t * NB : (t + 1) * NB].rearrange(
                            "(c p) j -> p c j", p=128
                        )
                        nc.sync.dma_start(xt, src)
                        pa = pp.tile([128, NB], f32, name=f"p0a_{t}", tag="pa")
                        pb = pp.tile([2, NB], f32, name=f"p0b_{t}", tag="pb")
                        for c in range(8):
                            nc.tensor.matmul(
                                pa,
                                w0a_sb[c],
                                xt[:, c * NB : (c + 1) * NB],
                                start=(c == 0),
                                stop=(c == 7),
                            )
                        for c in range(8):
                            nc.tensor.matmul(
                                pb,
                                w0b_sb[c],
                                xt[:, c * NB : (c + 1) * NB],
                                start=(c == 0),
                                stop=(c == 7),
                            )
                        za = zA.tile([128, NB], f32r, name=f"z0a_{t}", tag="zA")
                        nc.scalar.copy(za, pa)
                        z0a_tiles.append(za)
                        nc.vector.bn_stats(bn0a[:, 6 * t : 6 * t + 6], pa)
                        zb = xp.tile([2, NB], f32r, name=f"z0b_{t}", tag="z0bs", bufs=3)
                        nc.scalar.copy(zb, pb)
                        nc.vector.bn_stats(bn0b[:, 6 * t : 6 * t + 6], pb)
                        nc.sync.dma_start(z0b_dram[t], zb)

                    # stats + AllReduce for layer 0
                    compute_payload(sti[0], bn0a, ar_in[0][0:128, :])
                    compute_payload(st0b, bn0b, ar_in[0][128:130, :])
                    nc.gpsimd.collective_compute(
                        "AllReduce",
                        OP.add,
                        replica_groups=rg,
                        ins=[ar_in[0].opt()],
                        outs=[ar_out[0].opt()],
                    )
                    postprocess_stats(
                        sti[0], g_sb[0][0], g_sb[0][1], be_sb[0], ar_out[0][0:128, :]
                    )
                    postprocess_stats(st0b, g0b, gi0b, be0b, ar_out[0][128:130, :])
                    # fold BN scale into W1
                    nc.vector.tensor_scalar_mul(wf1a, w1a_raw, sti[0]["a"])
                    nc.vector.tensor_scalar_mul(wf1b, w1b_raw, st0b["a"])

                # ================= phases 1..4 =================
                prev_a = z0a_tiles
                cur_tiles = None
                for li in range(1, 5):
                    d = HID[li]
                    zp = zB if li % 2 == 1 else zA
                    ztag = "zB" if li % 2 == 1 else "zA"
                    cur_tiles = []
                    for t in range(NT):
                        # relu(z + nb) in place on prev layer tile
                        if li == 1:
                            nc.vector.tensor_scalar(
                                prev_a[t],
                                prev_a[t],
                                sti[0]["nb"],
                                0.0,
                                op0=OP.add,
                                op1=OP.max,
                            )
                            hb = wp.tile(
                                [2, NB], f32r, name=f"h0b_{t}", tag="h0b", bufs=4
                            )
                            nc.sync.dma_start(hb, z0b_dram[t])
                            nc.vector.tensor_scalar(
                                hb, hb, st0b["nb"], 0.0, op0=OP.add, op1=OP.max
                            )
                            ps = pp.tile([d, NB], f32, name=f"p1_{t}", tag="pa")
                            nc.tensor.matmul(ps, wf1a, prev_a[t], start=True, stop=False)
                            nc.tensor.matmul(ps, wf1b, hb, start=False, stop=True)
                        else:
                            nc.vector.tensor_scalar(
                                prev_a[t],
                                prev_a[t],
                                sti[li - 1]["nb"],
                                0.0,
                                op0=OP.add,
                                op1=OP.max,
                            )
                            ps = pp.tile([d, NB], f32, name=f"p{li}_{t}", tag="pa")
                            nc.tensor.matmul(
                                ps, wf[li], prev_a[t], start=True, stop=True
                            )
                        zt = zp.tile([d, NB], f32r, name=f"z{li}_{t}", tag=ztag)
                        nc.scalar.copy(zt, ps)
                        cur_tiles.append(zt)
                        nc.vector.bn_stats(bn_buf[li][:, 6 * t : 6 * t + 6], ps)

                    compute_payload(sti[li], bn_buf[li], ar_in[li][:, :])
                    nc.gpsimd.collective_compute(
                        "AllReduce",
                        OP.add,
                        replica_groups=rg,
                        ins=[ar_in[li].opt()],
                        outs=[ar_out[li].opt()],
                    )
                    postprocess_stats(
                        sti[li], g_sb[li][0], g_sb[li][1], be_sb[li], ar_out[li][:, :]
                    )
                    if li < 4:
                        nc.vector.tensor_scalar_mul(wf[li + 1], w_raw[li + 1], sti[li]["a"])
                    else:
                        nc.vector.tensor_scalar_mul(wf5, w5_raw, sti[4]["a"])
                    prev_a = cur_tiles

                # ================= phase 5: final linear + sampling =================
                with tc.tile_pool(name="fp", bufs=3) as fp:
                    for t in range(NT):
                        nc.vector.tensor_scalar(
                            prev_a[t],
                            prev_a[t],
                            sti[4]["nb"],
                            0.0,
                            op0=OP.add,
                            op1=OP.max,
                        )
                        ps = pp.tile([W, NB], f32, name=f"p5_{t}", tag="pa")
                        nc.tensor.matmul(ps, wf5, prev_a[t], start=True, stop=True)
                        ut = fp.tile([W, NB], f32, name=f"u_{t}", tag="u")
                        nc.sync.dma_start(ut, uT.ap()[:, t * NB : (t + 1) * NB])
                        q = fp.tile([W, NB], f32, name=f"q_{t}", tag="q")
                        # q = sigmoid(-(z5 + b5)) = 1 - p
                        nc.scalar.activation(q, ps, AF.Sigmoid, bias=b5neg, scale=-1.0)
                        ce = fp.tile([W, NB], f32, name=f"ce_{t}", tag="ce")
                        nc.vector.tensor_tensor(ce, ut, q, op=OP.is_ge)
                        nc.sync.dma_start(chosenT.ap()[:, t * NB : (t + 1) * NB], ce)
                        dd = fp.tile([W, NB], f32, name=f"d_{t}", tag="d")
                        nc.vector.tensor_tensor(dd, ce, q, op=OP.subtract)
                        nc.vector.tensor_tensor(dd, dd, dd, op=OP.mult)
                        lg = fp.tile([W, NB], f32r, name=f"lg_{t}", tag="lg")
                        nc.scalar.activation(lg, dd, AF.Ln)
                        pl = pp.tile([1, NB], f32, name=f"pl_{t}", tag="pb")
                        nc.tensor.matmul(pl, ones_sb, lg, start=True, stop=True)
                        lp = fp.tile([1, NB], f32, name=f"lp_{t}", tag="lp")
                        nc.scalar.copy(lp, pl)
                        nc.sync.dma_start(logp.ap()[:, t * NB : (t + 1) * NB], lp)

    nc.compile()
    return nc


def _get_program():
    if "nc" not in _prog_cache:
        _prog_cache["nc"] = _build_program()
    return _prog_cache["nc"]


def _compute_uniforms():
    import jax
    import jax.numpy as jnp

    cpu = jax.devices("cpu")[0]
    with jax.default_device(cpu):
        u = jax.random.uniform(jax.random.key(42), (B, W), dtype=jnp.float32)
        return np.asarray(u)


def kernel(**inputs):
    global last_results
    from concourse import bass_utils

    events = np.ascontiguousarray(np.asarray(inputs["events"], dtype=np.float32))
    x = events.reshape(B, F)
    u = _compute_uniforms()

    weights = {}
    weights["w0T"] = np.ascontiguousarray(np.asarray(inputs["W0"], np.float32).T)
    for i in range(1, 5):
        weights[f"w{i}T"] = np.ascontiguousarray(
            np.asarray(inputs[f"W{i}"], np.float32).T
        )
    weights["w5T"] = np.ascontiguousarray(np.asarray(inputs["W5"], np.float32).T)
    weights["b5"] = np.ascontiguousarray(np.asarray(inputs["b5"], np.float32))
    for i in range(5):
        weights[f"g{i}"] = np.ascontiguousarray(np.asarray(inputs[f"g{i}"], np.float32))
        weights[f"be{i}"] = np.ascontiguousarray(
            np.asarray(inputs[f"be{i}"], np.float32)
        )

    in_maps = []
    for c in range(NCORES):
        m = dict(weights)
        m["xT"] = np.ascontiguousarray(x[c * BC : (c + 1) * BC, :].T)
        m["uT"] = np.ascontiguousarray(u[c * BC : (c + 1) * BC, :].T)
        in_maps.append(m)

    nc = _get_program()
    trace = os.environ.get("KERNEL_TRACE", "0") == "1"
    res = bass_utils.run_bass_kernel_spmd(
        nc,
        in_maps,
        core_ids=list(range(NCORES)),
        trace=trace,
    )
    last_results = res

    chosen = np.empty((B, W), dtype=np.float32)
    log_probs = np.empty((B,), dtype=np.float32)
    for c in range(NCORES):
        r = res.results[c]
        chosen[c * BC : (c + 1) * BC, :] = r["chosenT"].T
        log_probs[c * BC : (c + 1) * BC] = r["logp"].reshape(-1)
    return chosen, log_probs
